# revision 32
# baseline (speedup 1.0000x reference)
"""Trainium2 Bass kernel: low-rank (LoRA-style) linear with 2:4 soft-threshold
pruned weights, fp16 matmul / fp32 accumulate.

  wA = soft_threshold24(weight_A) * scale_A          # [IN, R]
  wB = soft_threshold24(weight_B) * scale_B          # [OUT, R]
  x_proj = f16(x) @ f16(wA)            (f32 accum)   # [N, R]
  out    = f16(x_proj) @ f16(wB).T + bias            # [N, OUT]

Sharding: data-parallel over the token dim across 8 cores (2048 tokens/core),
small weights replicated. No collectives.

Host does dtype/layout prep only (f16 cast, transpose, row packing); all of
the module's math (threshold, both matmuls, bias) runs on device.

Structure (v8): the host uploads each core's x shard transposed and packed
so every DMA tile is [128, 4096] f16 (8KB partition rows = full ~420 GB/s;
narrow rows are descriptor-bound ~270).  Tokens are processed in 4 chunks of
512; the PE FIFO runs mm1(A) -> transposes(wB.T) -> [mm2(c) with mm1(c+1)
matmuls woven into the drain-stall slots] -> ... -> mm2(D).  Chunk stores
overlap later chunks' compute on the Pool/SP DGE queues.  PSUM: 2 banks
ping-pong the x_proj accumulators, 6 banks give mm2 three [128,1024]
pair-drain buffers (one ACT/DVE copy moves two matmuls' output).  The 2:4
soft-threshold runs on DVE with abs_max + per-group clips (no full-width
temporaries); wA/wB are each done in halves so consumers unblock early.
"""

import sys

import numpy as np

if "/opt/trn_rl_repo" not in sys.path:
    sys.path.insert(0, "/opt/trn_rl_repo")

B, S, IN_F, OUT_F, RANK = 4, 4096, 4096, 4096, 64
N_CORES = 8
N_TOK = B * S                   # 16384
T_CORE = N_TOK // N_CORES       # 2048 tokens per core
P = 128
N_K = IN_F // P                 # 32 contraction tiles
MM2_N = 512
N_OB = OUT_F // MM2_N           # 8 output column groups

CH = 4                          # token chunks (pipelined)
CTOK = T_CORE // CH             # 512 tokens per chunk
KQ = 8                          # k-rows packed per load tile
NG = N_K // KQ                  # 4 load tiles per chunk
CTT = CTOK // P                 # 4 mm2 token tiles per chunk

_CACHE = {}


def _soft_threshold(nc, pool, wfh, nb, scale, out_ap, pfx):
    """soft_threshold24(wfh)*scale -> out_ap (f16), on DVE.

    wfh: [P, nb, RANK] f16 AP (host-permuted rows; the threshold is
    elementwise over groups of 4 along R, so row order is free).
    t = 2nd-smallest |.| of each group of 4; out = w - clip(w, -t, t).
    All ops are group-strided (RANK/4 wide) -- no full-width temporaries.
    """
    import concourse.mybir as mybir

    f16 = mybir.dt.float16
    amin, amx = mybir.AluOpType.min, mybir.AluOpType.max
    ve = nc.vector

    g = wfh.rearrange("p b (g q) -> p b g q", q=4)
    og = out_ap.rearrange("p b (g q) -> p b g q", q=4)
    gj = [g[:, :, :, j : j + 1] for j in range(4)]
    ash = [P, nb, RANK // 4, 1]
    wneg = pool.tile([P, nb, RANK], f16, tag=pfx + "wneg", name="wneg")
    ve.tensor_scalar_mul(wneg[:], wfh, -1.0)
    ng = wneg[:].rearrange("p b (g q) -> p b g q", q=4)
    ab = [pool.tile(ash, f16, tag=f"{pfx}ab{j}", name=f"ab{j}")
          for j in range(4)]
    for j in range(4):
        ve.tensor_tensor(ab[j][:], gj[j], ng[:, :, :, j : j + 1], op=amx)
    m1 = pool.tile(ash, f16, tag=pfx + "m1", name="m1")
    M1 = pool.tile(ash, f16, tag=pfx + "M1", name="M1")
    m2 = pool.tile(ash, f16, tag=pfx + "m2", name="m2")
    M2 = pool.tile(ash, f16, tag=pfx + "M2", name="M2")
    ve.tensor_tensor(m1[:], ab[0][:], ab[1][:], op=amin)
    ve.tensor_tensor(M1[:], ab[0][:], ab[1][:], op=amx)
    ve.tensor_tensor(m2[:], ab[2][:], ab[3][:], op=amin)
    ve.tensor_tensor(M2[:], ab[2][:], ab[3][:], op=amx)
    # t = 2nd smallest = min(max(m1, m2), min(M1, M2))
    t = pool.tile(ash, f16, tag=pfx + "t", name="t")
    ve.tensor_tensor(m1[:], m1[:], m2[:], op=amx)
    ve.tensor_tensor(M1[:], M1[:], M2[:], op=amin)
    ve.tensor_tensor(t[:], m1[:], M1[:], op=amin)
    nt = pool.tile(ash, f16, tag=pfx + "nt", name="nt")
    ve.tensor_scalar_mul(nt[:], t[:], -1.0)
    # out_j = g_j - clip(g_j, -t, t), per group lane (ab_j reused as tmp)
    for j in range(4):
        ve.tensor_tensor(ab[j][:], gj[j], t[:], op=amin)
        ve.tensor_tensor(ab[j][:], ab[j][:], nt[:], op=amx)
        ve.tensor_sub(og[:, :, :, j : j + 1], gj[j], ab[j][:])
    if scale != 1.0:
        ve.tensor_scalar_mul(out_ap, out_ap, float(scale))


def _build(scale_a, scale_b):
    import concourse.mybir as mybir
    import concourse.tile as tile
    from concourse import bacc
    from concourse.bass import ts
    from concourse.masks import make_identity

    f32, f16 = mybir.dt.float32, mybir.dt.float16

    nc = bacc.Bacc("TRN2", target_bir_lowering=False, debug=False,
                   enable_asserts=False)
    # packed transposed x: row (c*NG+g)*128+p, col kk*CTOK+t
    #   = f16(x)[core, c*CTOK+t, (KQ*g+kk)*128+p]
    x_d = nc.dram_tensor("x", [T_CORE, IN_F], f16, kind="ExternalInput")
    wa_d = nc.dram_tensor("weight_A", [IN_F, RANK], f16, kind="ExternalInput")
    wb_d = nc.dram_tensor("weight_B", [OUT_F, RANK], f16, kind="ExternalInput")
    b_d = nc.dram_tensor("bias", [1, OUT_F], f16, kind="ExternalInput")
    o_d = nc.dram_tensor("out", [T_CORE, OUT_F], f16, kind="ExternalOutput")

    with tile.TileContext(nc) as tc:
        with (
            tc.tile_pool(name="const", bufs=1) as constp,
            tc.tile_pool(name="wtmp", bufs=1) as wtmp,
            tc.tile_pool(name="bulk", bufs=8) as bulkp,
            tc.tile_pool(name="outp", bufs=6) as outp,
            tc.tile_pool(name="proj", bufs=1) as projp,
            tc.tile_pool(name="ps1", bufs=1, space="PSUM") as ps1p,
            tc.tile_pool(name="ps2", bufs=3, space="PSUM") as ps2p,
        ):
            ident16 = constp.tile([P, P], f16)
            make_identity(nc, ident16[:])

            # --- weight staging DMAs first on the SP queue (the DVE
            # threshold chain gates mm1/mm2 starts) ---
            NB_B = OUT_F // P
            wfa = wtmp.tile([P, N_K, RANK], f16, tag="awstage", name="wfa")
            nc.sync.dma_start(wfa[:],
                              wa_d[:].rearrange("(c n) r -> c n r", c=P))
            wfb = wtmp.tile([P, NB_B, RANK], f16, tag="bwstage", name="wfb")
            nc.sync.dma_start(wfb[:],
                              wb_d[:].rearrange("(c n) r -> c n r", c=P))

            # --- x loads on the SP queue: 16 tiles [128, 4096] f16 ---
            xts = [[None] * NG for _ in range(CH)]
            for c in range(CH):
                for g in range(NG):
                    bt = bulkp.tile([P, KQ * CTOK], f16, name="bulk",
                                    tag="bulk")
                    nc.sync.dma_start(bt[:], x_d[ts(c * NG + g, P), :])
                    xts[c][g] = bt

            # --- thresholds on DVE, each weight in two halves so
            # consumers unblock at half-chain latency; wA first ---
            HK = N_K // 2
            wa16h = [constp.tile([P, HK, RANK], f16, tag=f"wa16h{h}",
                                 name=f"wa16h{h}") for h in range(2)]
            for h in range(2):
                _soft_threshold(nc, wtmp, wfa[:, h * HK : (h + 1) * HK, :],
                                HK, scale_a, wa16h[h][:], "a")
            HB = NB_B // 2
            thr_bh = [wtmp.tile([P, HB, RANK], f16, tag=f"bthr{h}",
                                name=f"bthr{h}") for h in range(2)]
            for h in range(2):
                _soft_threshold(nc, wtmp, wfb[:, h * HB : (h + 1) * HB, :],
                                HB, scale_b, thr_bh[h][:], "b")
            wbt = constp.tile([RANK + 1, OUT_F], f16)  # wB.T (+ bias row)
            nc.scalar.dma_start(wbt[RANK : RANK + 1, :], b_d[:])

            # x_proj f16 staging (+ones row for the bias trick)
            xpa = projp.tile([RANK + 1, T_CORE], f16)
            nc.gpsimd.memset(xpa[RANK : RANK + 1, :], 1.0)

            def drain(it, n=None):
                if it is None:
                    return None
                try:
                    if n is None:
                        while True:
                            next(it)
                    else:
                        for _ in range(n):
                            next(it)
                except StopIteration:
                    return None
                return it

            def mm1_quanta(c, acc):
                """Per-k mm1 quanta for chunk c: acc[64, 512] += wa_k^T x."""
                for k in range(N_K):
                    g, kk = k // KQ, k % KQ
                    nc.tensor.matmul(acc[:], wa16h[k // HK][:, k % HK, :],
                                     xts[c][g][:, ts(kk, CTOK)],
                                     start=(k == 0), stop=(k == N_K - 1))
                    yield

            def cast_xproj(c, acc):
                if c % 2 == 0:
                    nc.scalar.copy(xpa[0:RANK, ts(c, CTOK)], acc[:])
                else:
                    nc.vector.tensor_copy(xpa[0:RANK, ts(c, CTOK)], acc[:])

            def mm2_chunk(c, fill):
                """mm2 for chunk c: [128,1024] PSUM pair-drains on ACT/DVE;
                `fill` (next chunk's mm1) absorbs PE drain-stall slots."""
                for t in range(CTT):
                    tt = c * CTT + t
                    ob = outp.tile([P, OUT_F], f16, name="ob", tag="ob")
                    for jp in range(N_OB // 2):
                        ps2 = ps2p.tile([P, 2 * MM2_N], f32, tag="ps2",
                                        name="ps2")
                        for h in range(2):
                            nc.tensor.matmul(ps2[:, ts(h, MM2_N)],
                                             xpa[:, ts(tt, P)],
                                             wbt[:, ts(2 * jp + h, MM2_N)],
                                             start=True, stop=True)
                            fill = drain(fill, 1)
                        dst = ob[:, ts(jp, 2 * MM2_N)]
                        if jp % 2 == 0:
                            nc.vector.tensor_copy(dst, ps2[:])
                        else:
                            nc.scalar.copy(dst, ps2[:])
                    # stores: Pool DGE queue; last chunk splits onto SP
                    # (its loads are long done)
                    if c == CH - 1 and t % 2 == 1:
                        nc.sync.dma_start(o_d[ts(tt, P), :], ob[:])
                    else:
                        nc.gpsimd.dma_start(o_d[ts(tt, P), :], ob[:])
                return fill

            # --- PE FIFO: mm1(A); wbt transposes; then per chunk:
            # mm2(c) with mm1(c+1) woven in ---
            accs = [None] * CH
            accs[0] = ps1p.tile([RANK, CTOK], f32, tag="acc0", name="acc0")
            drain(mm1_quanta(0, accs[0]))

            for b in range(NB_B):
                pw = ps2p.tile([P, 2 * MM2_N], f32, tag="ps2", name="pw")
                pwv = pw[0:RANK, 0 : P // 2].bitcast(f16)
                nc.tensor.transpose(pwv, thr_bh[b // HB][:, b % HB, :],
                                    ident16[:])
                if b % 2 == 0:
                    nc.scalar.copy(wbt[0:RANK, ts(b, P)], pwv)
                else:
                    nc.vector.tensor_copy(wbt[0:RANK, ts(b, P)], pwv)
            cast_xproj(0, accs[0])

            fill = None
            for c in range(CH):
                if c + 1 < CH:
                    accs[c + 1] = ps1p.tile([RANK, CTOK], f32,
                                            tag=f"acc{(c + 1) % 2}",
                                            name="accn")
                    fill = mm1_quanta(c + 1, accs[c + 1])
                else:
                    fill = None
                fill = mm2_chunk(c, fill)
                drain(fill)
                if c + 1 < CH:
                    cast_xproj(c + 1, accs[c + 1])

    nc.compile()
    return nc


def get_nc(scale_a, scale_b):
    key = (float(scale_a), float(scale_b))
    if key not in _CACHE:
        _CACHE[key] = _build(*key)
    return _CACHE[key]


def make_in_maps(x, weight_A, weight_B, bias):
    """Host-side shard + f16 cast + transpose/pack: per-core input dicts."""
    x16 = np.asarray(x, dtype=np.float32).astype(np.float16)
    wa = np.asarray(weight_A, np.float32).astype(np.float16)
    wb = np.asarray(weight_B, np.float32).astype(np.float16)
    # Lossless row permutations so the device DMA is contiguous (4KB
    # descriptors instead of 128B row gathers):
    #   wa16[c, k, r] = wA[k*128+c, r] -> send rows in (c,k) order
    #   thr_b[p, b, r] = wB[b*128+p, r] -> send rows in (p,b) order
    wa = np.ascontiguousarray(
        wa.reshape(N_K, P, RANK).transpose(1, 0, 2).reshape(IN_F, RANK))
    wb = np.ascontiguousarray(
        wb.reshape(OUT_F // P, P, RANK).transpose(1, 0, 2)
        .reshape(OUT_F, RANK))
    bi = np.ascontiguousarray(
        np.asarray(bias, np.float32).astype(np.float16)).reshape(1, OUT_F)
    xf = x16.reshape(N_TOK, IN_F)

    def pack_x(core):
        # [IN, T] transposed shard, packed so each [128, 4096] device tile
        # holds KQ k-rows x CTOK chunk-tokens per partition (8KB DMA rows):
        #   x_d[(c*NG+g)*128+p, kk*CTOK+t] = xT[(KQ*g+kk)*128+p, c*CTOK+t]
        xt = xf[core * T_CORE : (core + 1) * T_CORE].T  # [IN_F, T_CORE]
        a = xt.reshape(NG, KQ, P, CH, CTOK)             # [g, kk, p, c, t]
        a = a.transpose(3, 0, 2, 1, 4)                  # [c, g, p, kk, t]
        return np.ascontiguousarray(a.reshape(T_CORE, IN_F))

    return [
        {
            "x": pack_x(c),
            "weight_A": wa,
            "weight_B": wb,
            "bias": bi,
        }
        for c in range(N_CORES)
    ]


def kernel(x, weight_A, weight_B, bias, scale_A, scale_B):
    from concourse.bass_utils import run_bass_kernel_spmd

    sa = float(np.asarray(scale_A))
    sb = float(np.asarray(scale_B))
    nc = get_nc(sa, sb)

    in_maps = make_in_maps(x, weight_A, weight_B, bias)
    res = run_bass_kernel_spmd(nc, in_maps, core_ids=list(range(N_CORES)))
    out = np.concatenate([r["out"] for r in res.results], axis=0)
    return out.astype(np.float32).reshape(B, S, OUT_F)
